# revision 9
# baseline (speedup 1.0000x reference)
"""Trainium2 Bass kernel for dynamic-scale FP8 GEMM (MixLinear):

    out = (scale_in * scale_w) * (q8(x / scale_in) @ q8(w).T) + bias
    scale_in = max|x| / 448  (global over the whole activation tensor)

Strategy (8 NeuronCores, SPMD):
  - Data-parallel over M = B*S = 16384: each core gets a 2048-row shard of x,
    full weight + bias (replicated).
  - On-device global amax: per-core abs-max reduce, then AllReduce(max).
  - TRN fp8_e4m3 saturates at +-240 (vs OCP e4m3fn's +-448), so quantize with
    a 2x scale (values land in +-224) and fold the 2x back at dequant time.
  - x and w are DMA-transposed (fp16, xbar path) into [K-partition, K/128, M|N]
    layout, quantized on-chip to fp8, and the GEMM runs in DoubleRow perf mode
    (contraction 256 per matmul).
  - PSUM is evicted with a single ScalarE activation: out = psum*2s + bias
    (output kept N-major: psum partitions = N-tile), so bias is a per-partition
    scalar.  Per-core output is [N, M_shard]; the host transposes on gather.
"""

import os
import sys

try:
    import concourse  # noqa: F401
except ImportError:  # pragma: no cover
    for _p in ("/opt/trn_rl_repo", "/root/.axon_site/_ro/trn_rl_repo"):
        if os.path.isdir(_p) and _p not in sys.path:
            sys.path.insert(0, _p)

import numpy as np

import concourse.bacc as bacc
import concourse.bass as bass  # noqa: F401
import concourse.mybir as mybir
import concourse.tile as tile
from concourse import bass_isa
from concourse.bass_utils import run_bass_kernel_spmd

# Problem shapes (hardcoded per contract).
B, S, K, N = 4, 4096, 2048, 2048
M = B * S
N_CORES = 8
MS = M // N_CORES  # 2048 rows of x per core

P = 128
F16 = mybir.dt.float16
F32 = mybir.dt.float32
FP8 = mybir.dt.float8e4


def build_nc(ms=MS, k=K, n=N, n_cores=N_CORES):
    """Build + compile the per-core Bass program (SPMD: same NEFF on all cores)."""
    ko = k // P          # k-outer planes
    assert k % 256 == 0 and ms % 1024 == 0 and n % 256 == 0
    m_chunks = ms // 256     # x load/quant chunk count
    n_chunks = n // 256      # w load/quant chunk count
    nt_tiles = n // P        # GEMM stationary n-tiles
    k_pairs = ko // 2        # DoubleRow k steps
    m_half = ms // 2
    mc512 = m_half // 512    # 512-wide m chunks per half

    nc = bacc.Bacc("TRN2", target_bir_lowering=False, debug=False, num_devices=n_cores)
    x = nc.dram_tensor("x", [ms, k], F16, kind="ExternalInput")
    wt = nc.dram_tensor("wt", [k, n], F16, kind="ExternalInput")
    b = nc.dram_tensor("b", [n], F16, kind="ExternalInput")
    out_t = nc.dram_tensor("out_t", [n, ms], F16, kind="ExternalOutput")

    with tile.TileContext(nc) as tc:
        with (
            tc.tile_pool(name="big", bufs=1) as big,
            tc.tile_pool(name="xn", bufs=3) as xnp,
            tc.tile_pool(name="small", bufs=1) as small,
            tc.tile_pool(name="ev", bufs=4) as ev,
            tc.tile_pool(name="psum", bufs=2, space="PSUM") as psum,
            tc.tile_pool(name="dram", bufs=1, space="DRAM") as dram,
        ):
            # Persistent SBUF tensors.
            xT = big.tile([P, ko, ms], F16)    # x transposed, fp16
            xq = big.tile([P, ko, ms], FP8)    # quantized x (scale 2s)
            wq = big.tile([P, ko, n], FP8)     # quantized w (scale 1)

            # ---- Phase A ------------------------------------------------
            # amax path: natural-layout x chunk loads (plain DMA, ScalarE
            # HWDGE queue) feeding DVE abs-max reduces.  In parallel the Sync
            # queue runs the 8 xbar transposes of x (xbar transposes corrupt
            # if issued on two queues concurrently -- keep them on one).
            acc_cols = small.tile([P, m_chunks * 2], F32)
            for mc in range(m_chunks):
                xnat = xnp.tile([P, 2, k], F16, tag="xn", name=f"xn_{mc}")
                nc.scalar.dma_start(
                    out=xnat[:],
                    in_=x.ap()[mc * 256:(mc + 1) * 256, :].rearrange(
                        "(a p) c -> p a c", p=P
                    ),
                )
                nc.vector.tensor_reduce(
                    acc_cols[:, mc * 2:(mc + 1) * 2],
                    xnat[:],
                    axis=mybir.AxisListType.X,
                    op=mybir.AluOpType.max,
                    apply_absolute_value=True,
                )
            for mc in range(m_chunks):
                nc.sync.dma_start(
                    out=xT[:, :, mc * 256:(mc + 1) * 256],
                    in_=x.ap()[mc * 256:(mc + 1) * 256, :],
                    transpose=True,
                )
            amax_col = small.tile([P, 1], F32)
            nc.vector.tensor_reduce(
                amax_col, acc_cols[:], axis=mybir.AxisListType.X,
                op=mybir.AluOpType.max,
            )
            amax_all = small.tile([P, 1], F32)
            nc.gpsimd.partition_all_reduce(
                amax_all, amax_col, channels=P, reduce_op=bass_isa.ReduceOp.max
            )

            # ---- AllGather amaxes across cores, reduce locally ------------
            cc_in = dram.tile([1], F32)
            cc_addr = "Shared" if n_cores > 4 else "Local"
            cc_out = dram.tile([n_cores], F32, addr_space=cc_addr)
            nc.scalar.dma_start(cc_in[:], amax_all[0:1, 0])
            nc.gpsimd.collective_compute(
                "AllGather",
                mybir.AluOpType.bypass,
                replica_groups=[list(range(n_cores))],
                ins=[cc_in.opt()],
                outs=[cc_out.opt()],
            )
            # ---- Phase W: weight load+quantize in one SWDGE cast-DMA ------
            # wt DRAM is already [K, N]; SWDGE casts fp16->fp8e4 (RNE,
            # verified) during the transfer.  k = j*128 + p matches the
            # xbar-transpose k-mapping used for x.
            nc.gpsimd.dma_start(
                out=wq[:],
                in_=wt.ap().rearrange("(j p) n -> p j n", p=P),
            )

            # bias -> SBUF [128, n/128] fp32, [p, j] = bias[j*128 + p]
            bias16 = small.tile([P, nt_tiles], F16)
            nc.scalar.dma_start(bias16[:], b.ap().rearrange("(j p) -> p j", p=P))
            bias32 = small.tile([P, nt_tiles], F32)
            nc.vector.tensor_copy(bias32[:], bias16[:])

            scal0 = small.tile([P, n_cores], F32)
            nc.scalar.dma_start(scal0[0:1, :], cc_out[:])
            amax1 = small.tile([P, 1], F32)
            nc.vector.tensor_reduce(
                amax1[0:1, :], scal0[0:1, :], axis=mybir.AxisListType.X,
                op=mybir.AluOpType.max,
            )
            amax_bc = small.tile([P, 1], F32)
            nc.gpsimd.partition_broadcast(amax_bc, amax1[0:1, :], channels=P)

            # inv2s = 224/amax (quant scale), s2 = amax/224 (dequant scale)
            inv_amax = small.tile([P, 1], F32)
            nc.vector.reciprocal(inv_amax, amax_bc)
            inv2s = small.tile([P, 1], F32)
            nc.vector.tensor_scalar_mul(inv2s, inv_amax, 224.0)
            s2 = small.tile([P, 1], F32)
            nc.vector.tensor_scalar_mul(s2, amax_bc, 1.0 / 224.0)

            # ---- Phases Q+G interleaved: quantize a 512-m quarter, GEMM it.
            # Quantization alternates ScalarE (activation w/ scale) and
            # VectorE (tensor_scalar mult) so a quarter is ready in ~4us.
            for mq in range(ms // 512):
                for h in range(2):
                    mc = 2 * mq + h
                    sl = slice(mc * 256, (mc + 1) * 256)
                    if h == 0:
                        nc.scalar.activation(
                            xq[:, :, sl], xT[:, :, sl],
                            mybir.ActivationFunctionType.Copy, scale=inv2s[:],
                        )
                    else:
                        nc.vector.tensor_scalar(
                            xq[:, :, sl], xT[:, :, sl], inv2s[:], None,
                            mybir.AluOpType.mult,
                        )
                m0 = mq * 512
                for nt in range(nt_tiles):
                    ps = psum.tile(
                        [P, 512], F32, tag="ps", bufs=4, name=f"ps_{mq}_{nt}"
                    )
                    for k8 in range(k_pairs):
                        nc.tensor.matmul(
                            ps[:],
                            lhsT=wq[:, 2 * k8:2 * k8 + 2, nt * P:(nt + 1) * P],
                            rhs=xq[:, 2 * k8:2 * k8 + 2, m0:m0 + 512],
                            start=(k8 == 0),
                            stop=(k8 == k_pairs - 1),
                            perf_mode=mybir.MatmulPerfMode.DoubleRow,
                        )
                    ob = ev.tile([P, 512], F16, tag="ob", name=f"ob_{mq}_{nt}")
                    nc.scalar.activation(
                        ob[:], ps[:],
                        mybir.ActivationFunctionType.Identity,
                        bias=bias32[:, nt:nt + 1],
                        scale=s2[:],
                    )
                    nc.scalar.dma_start(
                        out_t.ap()[nt * P:(nt + 1) * P, m0:m0 + 512], ob[:]
                    )

    nc.compile()
    return nc


_NC_CACHE = {}


def _get_nc():
    if "nc" not in _NC_CACHE:
        _NC_CACHE["nc"] = build_nc()
    return _NC_CACHE["nc"]


def kernel(x, weight, bias):
    x = np.asarray(x, dtype=np.float16).reshape(M, K)
    weight = np.asarray(weight, dtype=np.float16)
    bias = np.asarray(bias, dtype=np.float16)

    nc = _get_nc()
    wt = np.ascontiguousarray(weight.T)  # [K, N] — static-weight layout prep
    in_maps = [
        {"x": x[c * MS:(c + 1) * MS], "wt": wt, "b": bias}
        for c in range(N_CORES)
    ]
    trace = bool(int(os.environ.get("KERNEL_TRACE", "0")))
    res = run_bass_kernel_spmd(nc, in_maps, list(range(N_CORES)), trace=trace)
    _NC_CACHE["last_result"] = res

    out = np.empty((M, N), dtype=np.float16)
    for c in range(N_CORES):
        out[c * MS:(c + 1) * MS, :] = res.results[c]["out_t"].T
    return out.reshape(B, S, N)


# revision 12
# speedup vs baseline: 1.2310x; 1.2310x over previous
"""Trainium2 Bass kernel for dynamic-scale FP8 GEMM (MixLinear):

    out = (scale_in * scale_w) * (q8(x / scale_in) @ q8(w).T) + bias
    scale_in = max|x| / 448  (global over the whole activation tensor)

Strategy (8 NeuronCores, SPMD):
  - Data-parallel over M = B*S = 16384: each core gets a 2048-row shard of x,
    full weight + bias (replicated).
  - On-device global amax: per-core abs-max reduce, then AllReduce(max).
  - TRN fp8_e4m3 saturates at +-240 (vs OCP e4m3fn's +-448), so quantize with
    a 2x scale (values land in +-224) and fold the 2x back at dequant time.
  - x and w are DMA-transposed (fp16, xbar path) into [K-partition, K/128, M|N]
    layout, quantized on-chip to fp8, and the GEMM runs in DoubleRow perf mode
    (contraction 256 per matmul).
  - PSUM is evicted with a single ScalarE activation: out = psum*2s + bias
    (output kept N-major: psum partitions = N-tile), so bias is a per-partition
    scalar.  Per-core output is [N, M_shard]; the host transposes on gather.
"""

import os
import sys

try:
    import concourse  # noqa: F401
except ImportError:  # pragma: no cover
    for _p in ("/opt/trn_rl_repo", "/root/.axon_site/_ro/trn_rl_repo"):
        if os.path.isdir(_p) and _p not in sys.path:
            sys.path.insert(0, _p)

import numpy as np

import concourse.bacc as bacc
import concourse.bass as bass  # noqa: F401
import concourse.mybir as mybir
import concourse.tile as tile
from concourse import bass_isa
from concourse.bass_utils import run_bass_kernel_spmd

# Problem shapes (hardcoded per contract).
B, S, K, N = 4, 4096, 2048, 2048
M = B * S
N_CORES = 8
MS = M // N_CORES  # 2048 rows of x per core

P = 128
F16 = mybir.dt.float16
F32 = mybir.dt.float32
FP8 = mybir.dt.float8e4


def build_nc(ms=MS, k=K, n=N, n_cores=N_CORES):
    """Build + compile the per-core Bass program (SPMD: same NEFF on all cores)."""
    ko = k // P          # k-outer planes
    assert k % 256 == 0 and ms % 1024 == 0 and n % 256 == 0
    m_chunks = ms // 256     # x load/quant chunk count
    n_chunks = n // 256      # w load/quant chunk count
    nt_tiles = n // P        # GEMM stationary n-tiles
    k_pairs = ko // 2        # DoubleRow k steps
    m_half = ms // 2
    mc512 = m_half // 512    # 512-wide m chunks per half

    nc = bacc.Bacc("TRN2", target_bir_lowering=False, debug=False, num_devices=n_cores)
    x = nc.dram_tensor("x", [ms, k], F16, kind="ExternalInput")
    wt = nc.dram_tensor("wt", [k, n], F16, kind="ExternalInput")
    b = nc.dram_tensor("b", [n], F16, kind="ExternalInput")
    out_t = nc.dram_tensor("out_t", [n, ms], F16, kind="ExternalOutput")

    with tile.TileContext(nc) as tc:
        with (
            tc.tile_pool(name="big", bufs=1) as big,
            tc.tile_pool(name="xn", bufs=3) as xnp,
            tc.tile_pool(name="small", bufs=1) as small,
            tc.tile_pool(name="ev", bufs=4) as ev,
            tc.tile_pool(name="psum", bufs=2, space="PSUM") as psum,
            tc.tile_pool(name="dram", bufs=1, space="DRAM") as dram,
        ):
            # Persistent SBUF tensors.
            xT = big.tile([P, m_chunks, ko, 256], F16)  # x^T, chunk-major
            xq = big.tile([P, ko, ms], FP8)    # quantized x (scale 2s)
            wq = big.tile([P, ko, n], FP8)     # quantized w (scale 1)

            # ---- Phase A ------------------------------------------------
            # Xbar transposes of x, back-to-back on the Sync queue (xbar
            # transposes corrupt when issued from two queues concurrently,
            # and Tile serializes transpose<->copy transitions globally --
            # so keep the burst clean).  Chunk-major destination keeps each
            # transpose's S2M side fully contiguous per partition.
            for mc in range(m_chunks):
                nc.sync.dma_start(
                    out=xT[:, mc],
                    in_=x.ap()[mc * 256:(mc + 1) * 256, :],
                    transpose=True,
                )
            # abs-max: per-chunk DVE reduces (apply_absolute_value)
            acc_cols = small.tile([P, m_chunks * 2], F32)
            for mc in range(m_chunks):
                nc.vector.tensor_reduce(
                    acc_cols[:, mc * 2:(mc + 1) * 2],
                    xT[:, mc].rearrange("p j f -> p (j f)").rearrange(
                        "p (a f) -> p a f", a=2
                    ),
                    axis=mybir.AxisListType.X,
                    op=mybir.AluOpType.max,
                    apply_absolute_value=True,
                )
            amax_col = small.tile([P, 1], F32)
            nc.vector.tensor_reduce(
                amax_col, acc_cols[:], axis=mybir.AxisListType.X,
                op=mybir.AluOpType.max,
            )
            amax_all = small.tile([P, 1], F32)
            nc.gpsimd.partition_all_reduce(
                amax_all, amax_col, channels=P, reduce_op=bass_isa.ReduceOp.max
            )

            # ---- AllGather amaxes across cores, reduce locally ------------
            cc_in = dram.tile([1], F32)
            cc_addr = "Shared" if n_cores > 4 else "Local"
            cc_out = dram.tile([n_cores], F32, addr_space=cc_addr)
            nc.scalar.dma_start(cc_in[:], amax_all[0:1, 0])
            nc.gpsimd.collective_compute(
                "AllGather",
                mybir.AluOpType.bypass,
                replica_groups=[list(range(n_cores))],
                ins=[cc_in.opt()],
                outs=[cc_out.opt()],
            )
            # ---- Phase W: weight load+quantize in one SWDGE cast-DMA ------
            # wt DRAM is already [K, N]; SWDGE casts fp16->fp8e4 (RNE,
            # verified) during the transfer.  k = j*128 + p matches the
            # xbar-transpose k-mapping used for x.
            nc.gpsimd.dma_start(
                out=wq[:],
                in_=wt.ap().rearrange("(j p) n -> p j n", p=P),
            )

            # bias -> SBUF [128, n/128] fp32, [p, j] = bias[j*128 + p]
            bias16 = small.tile([P, nt_tiles], F16)
            nc.scalar.dma_start(bias16[:], b.ap().rearrange("(j p) -> p j", p=P))
            bias32 = small.tile([P, nt_tiles], F32)
            nc.vector.tensor_copy(bias32[:], bias16[:])

            scal0 = small.tile([P, n_cores], F32)
            nc.scalar.dma_start(scal0[0:1, :], cc_out[:])
            amax1 = small.tile([P, 1], F32)
            nc.vector.tensor_reduce(
                amax1[0:1, :], scal0[0:1, :], axis=mybir.AxisListType.X,
                op=mybir.AluOpType.max,
            )
            amax_bc = small.tile([P, 1], F32)
            nc.gpsimd.partition_broadcast(amax_bc, amax1[0:1, :], channels=P)

            # inv2s = 224/amax (quant scale), s2 = amax/224 (dequant scale)
            inv_amax = small.tile([P, 1], F32)
            nc.vector.reciprocal(inv_amax, amax_bc)
            inv2s = small.tile([P, 1], F32)
            nc.vector.tensor_scalar_mul(inv2s, inv_amax, 224.0)
            s2 = small.tile([P, 1], F32)
            nc.vector.tensor_scalar_mul(s2, amax_bc, 1.0 / 224.0)

            # ---- Phases Q+G interleaved: quantize a 512-m quarter, GEMM it.
            # Quantization alternates ScalarE (activation w/ scale) and
            # VectorE (tensor_scalar mult) so a quarter is ready in ~4us.
            for mq in range(ms // 512):
                for h in range(2):
                    mc = 2 * mq + h
                    sl = slice(mc * 256, (mc + 1) * 256)
                    if h == 0:
                        nc.scalar.activation(
                            xq[:, :, sl], xT[:, mc],
                            mybir.ActivationFunctionType.Copy, scale=inv2s[:],
                        )
                    else:
                        nc.vector.tensor_scalar(
                            xq[:, :, sl], xT[:, mc], inv2s[:], None,
                            mybir.AluOpType.mult,
                        )
                m0 = mq * 512
                for nt in range(nt_tiles):
                    ps = psum.tile(
                        [P, 512], F32, tag="ps", bufs=4, name=f"ps_{mq}_{nt}"
                    )
                    for k8 in range(k_pairs):
                        nc.tensor.matmul(
                            ps[:],
                            lhsT=wq[:, 2 * k8:2 * k8 + 2, nt * P:(nt + 1) * P],
                            rhs=xq[:, 2 * k8:2 * k8 + 2, m0:m0 + 512],
                            start=(k8 == 0),
                            stop=(k8 == k_pairs - 1),
                            perf_mode=mybir.MatmulPerfMode.DoubleRow,
                        )
                    ob = ev.tile([P, 512], F16, tag="ob", name=f"ob_{mq}_{nt}")
                    nc.scalar.activation(
                        ob[:], ps[:],
                        mybir.ActivationFunctionType.Identity,
                        bias=bias32[:, nt:nt + 1],
                        scale=s2[:],
                    )
                    nc.sync.dma_start(
                        out_t.ap()[nt * P:(nt + 1) * P, m0:m0 + 512], ob[:]
                    )

    nc.compile()
    return nc


_NC_CACHE = {}


def _get_nc():
    if "nc" not in _NC_CACHE:
        _NC_CACHE["nc"] = build_nc()
    return _NC_CACHE["nc"]


def kernel(x, weight, bias):
    x = np.asarray(x, dtype=np.float16).reshape(M, K)
    weight = np.asarray(weight, dtype=np.float16)
    bias = np.asarray(bias, dtype=np.float16)

    nc = _get_nc()
    wt = np.ascontiguousarray(weight.T)  # [K, N] — static-weight layout prep
    in_maps = [
        {"x": x[c * MS:(c + 1) * MS], "wt": wt, "b": bias}
        for c in range(N_CORES)
    ]
    trace = bool(int(os.environ.get("KERNEL_TRACE", "0")))
    res = run_bass_kernel_spmd(nc, in_maps, list(range(N_CORES)), trace=trace)
    _NC_CACHE["last_result"] = res

    out = np.empty((M, N), dtype=np.float16)
    for c in range(N_CORES):
        out[c * MS:(c + 1) * MS, :] = res.results[c]["out_t"].T
    return out.reshape(B, S, N)


# revision 13
# speedup vs baseline: 1.2348x; 1.0031x over previous
"""Trainium2 Bass kernel for dynamic-scale FP8 GEMM (MixLinear):

    out = (scale_in * scale_w) * (q8(x / scale_in) @ q8(w).T) + bias
    scale_in = max|x| / 448  (global over the whole activation tensor)

Strategy (8 NeuronCores, SPMD):
  - Data-parallel over M = B*S = 16384: each core gets a 2048-row shard of x,
    full weight + bias (replicated).
  - On-device global amax: per-core abs-max reduce, then AllReduce(max).
  - TRN fp8_e4m3 saturates at +-240 (vs OCP e4m3fn's +-448), so quantize with
    a 2x scale (values land in +-224) and fold the 2x back at dequant time.
  - x and w are DMA-transposed (fp16, xbar path) into [K-partition, K/128, M|N]
    layout, quantized on-chip to fp8, and the GEMM runs in DoubleRow perf mode
    (contraction 256 per matmul).
  - PSUM is evicted with a single ScalarE activation: out = psum*2s + bias
    (output kept N-major: psum partitions = N-tile), so bias is a per-partition
    scalar.  Per-core output is [N, M_shard]; the host transposes on gather.
"""

import os
import sys

try:
    import concourse  # noqa: F401
except ImportError:  # pragma: no cover
    for _p in ("/opt/trn_rl_repo", "/root/.axon_site/_ro/trn_rl_repo"):
        if os.path.isdir(_p) and _p not in sys.path:
            sys.path.insert(0, _p)

import numpy as np

import concourse.bacc as bacc
import concourse.bass as bass  # noqa: F401
import concourse.mybir as mybir
import concourse.tile as tile
from concourse import bass_isa
from concourse.bass_utils import run_bass_kernel_spmd

# Problem shapes (hardcoded per contract).
B, S, K, N = 4, 4096, 2048, 2048
M = B * S
N_CORES = 8
MS = M // N_CORES  # 2048 rows of x per core

P = 128
F16 = mybir.dt.float16
F32 = mybir.dt.float32
FP8 = mybir.dt.float8e4


def build_nc(ms=MS, k=K, n=N, n_cores=N_CORES):
    """Build + compile the per-core Bass program (SPMD: same NEFF on all cores)."""
    ko = k // P          # k-outer planes
    assert k % 256 == 0 and ms % 1024 == 0 and n % 256 == 0
    m_chunks = ms // 256     # x load/quant chunk count
    n_chunks = n // 256      # w load/quant chunk count
    nt_tiles = n // P        # GEMM stationary n-tiles
    k_pairs = ko // 2        # DoubleRow k steps
    m_half = ms // 2
    mc512 = m_half // 512    # 512-wide m chunks per half

    nc = bacc.Bacc("TRN2", target_bir_lowering=False, debug=False, num_devices=n_cores)
    x = nc.dram_tensor("x", [ms, k], F16, kind="ExternalInput")
    wt = nc.dram_tensor("wt", [k, n], F16, kind="ExternalInput")
    b = nc.dram_tensor("b", [n], F16, kind="ExternalInput")
    out_t = nc.dram_tensor("out_t", [n, ms], F16, kind="ExternalOutput")

    with tile.TileContext(nc) as tc:
        with (
            tc.tile_pool(name="big", bufs=1) as big,
            tc.tile_pool(name="xn", bufs=3) as xnp,
            tc.tile_pool(name="small", bufs=1) as small,
            tc.tile_pool(name="ev", bufs=4) as ev,
            tc.tile_pool(name="psum", bufs=2, space="PSUM") as psum,
            tc.tile_pool(name="dram", bufs=1, space="DRAM") as dram,
        ):
            # Persistent SBUF tensors.
            xT = big.tile([P, m_chunks, ko, 256], F16)  # x^T, chunk-major
            xq = big.tile([P, ko, ms], FP8)    # quantized x (scale 2s)
            wq = big.tile([P, ko, n], FP8)     # quantized w (scale 1)

            # ---- Phase A ------------------------------------------------
            # 1) Natural-layout x chunk loads (plain DMA, ScalarE HWDGE
            #    queue) feed the DVE abs-max reduces -> early collective.
            # 2) Then the 8 xbar transposes of x run as one clean burst on
            #    Sync (xbar transposes corrupt when issued from two queues,
            #    and Tile serializes transpose<->copy transitions globally,
            #    so DRAM->SBUF copies are explicitly ordered around the
            #    burst with add_dep_helper).
            acc_cols = small.tile([P, m_chunks * 2], F32)
            nat_insts = []
            for mc in range(m_chunks):
                xnat = xnp.tile([P, 2, k], F16, tag="xn", name=f"xn_{mc}")
                ni = nc.scalar.dma_start(
                    out=xnat[:],
                    in_=x.ap()[mc * 256:(mc + 1) * 256, :].rearrange(
                        "(a p) c -> p a c", p=P
                    ),
                )
                nat_insts.append(ni)
                nc.vector.tensor_reduce(
                    acc_cols[:, mc * 2:(mc + 1) * 2],
                    xnat[:],
                    axis=mybir.AxisListType.X,
                    op=mybir.AluOpType.max,
                    apply_absolute_value=True,
                )
            tr_insts = []
            for mc in range(m_chunks):
                ti = nc.sync.dma_start(
                    out=xT[:, mc],
                    in_=x.ap()[mc * 256:(mc + 1) * 256, :],
                    transpose=True,
                )
                tile.add_dep_helper(
                    ti.ins, nat_insts[-1].ins,
                    reason="xbar: transpose burst after natural-load burst",
                )
                tr_insts.append(ti)
            amax_col = small.tile([P, 1], F32)
            nc.vector.tensor_reduce(
                amax_col, acc_cols[:], axis=mybir.AxisListType.X,
                op=mybir.AluOpType.max,
            )
            amax_all = small.tile([P, 1], F32)
            nc.gpsimd.partition_all_reduce(
                amax_all, amax_col, channels=P, reduce_op=bass_isa.ReduceOp.max
            )

            # ---- AllGather amaxes across cores, reduce locally ------------
            cc_in = dram.tile([1], F32)
            cc_addr = "Shared" if n_cores > 4 else "Local"
            cc_out = dram.tile([n_cores], F32, addr_space=cc_addr)
            nc.scalar.dma_start(cc_in[:], amax_all[0:1, 0])
            nc.gpsimd.collective_compute(
                "AllGather",
                mybir.AluOpType.bypass,
                replica_groups=[list(range(n_cores))],
                ins=[cc_in.opt()],
                outs=[cc_out.opt()],
            )
            # ---- Phase W: weight load+quantize via SWDGE cast-DMA ---------
            # wt DRAM is already [K, N]; SWDGE casts fp16->fp8e4 (RNE,
            # verified) during the transfer.  k = j*128 + p matches the
            # xbar-transpose k-mapping used for x.  Chunked so the GEMM can
            # start on the first n-range; ordered after the transpose burst
            # (DRAM->SBUF copies conflict with xbar-transpose mode).
            for i in range(4):
                n0 = i * (n // 4)
                wi = nc.gpsimd.dma_start(
                    out=wq[:, :, n0:n0 + n // 4],
                    in_=wt.ap()[:, n0:n0 + n // 4].rearrange(
                        "(j p) n2 -> p j n2", p=P
                    ),
                )
                tile.add_dep_helper(
                    wi.ins, tr_insts[-1].ins,
                    reason="xbar: weight copy after transpose burst",
                )

            # bias -> SBUF [128, n/128] fp32, [p, j] = bias[j*128 + p]
            bias16 = small.tile([P, nt_tiles], F16)
            bi = nc.scalar.dma_start(
                bias16[:], b.ap().rearrange("(j p) -> p j", p=P)
            )
            tile.add_dep_helper(
                bi.ins, tr_insts[-1].ins,
                reason="xbar: bias copy after transpose burst",
            )
            bias32 = small.tile([P, nt_tiles], F32)
            nc.vector.tensor_copy(bias32[:], bias16[:])

            scal0 = small.tile([P, n_cores], F32)
            nc.scalar.dma_start(scal0[0:1, :], cc_out[:])
            amax1 = small.tile([P, 1], F32)
            nc.vector.tensor_reduce(
                amax1[0:1, :], scal0[0:1, :], axis=mybir.AxisListType.X,
                op=mybir.AluOpType.max,
            )
            amax_bc = small.tile([P, 1], F32)
            nc.gpsimd.partition_broadcast(amax_bc, amax1[0:1, :], channels=P)

            # inv2s = 224/amax (quant scale), s2 = amax/224 (dequant scale)
            inv_amax = small.tile([P, 1], F32)
            nc.vector.reciprocal(inv_amax, amax_bc)
            inv2s = small.tile([P, 1], F32)
            nc.vector.tensor_scalar_mul(inv2s, inv_amax, 224.0)
            s2 = small.tile([P, 1], F32)
            nc.vector.tensor_scalar_mul(s2, amax_bc, 1.0 / 224.0)

            # ---- Phases Q+G interleaved: quantize a 512-m quarter, GEMM it.
            # Quantization alternates ScalarE (activation w/ scale) and
            # VectorE (tensor_scalar mult) so a quarter is ready in ~4us.
            for mq in range(ms // 512):
                for h in range(2):
                    mc = 2 * mq + h
                    sl = slice(mc * 256, (mc + 1) * 256)
                    if h == 0:
                        nc.scalar.activation(
                            xq[:, :, sl], xT[:, mc],
                            mybir.ActivationFunctionType.Copy, scale=inv2s[:],
                        )
                    else:
                        nc.vector.tensor_scalar(
                            xq[:, :, sl], xT[:, mc], inv2s[:], None,
                            mybir.AluOpType.mult,
                        )
                m0 = mq * 512
                for nt in range(nt_tiles):
                    ps = psum.tile(
                        [P, 512], F32, tag="ps", bufs=4, name=f"ps_{mq}_{nt}"
                    )
                    for k8 in range(k_pairs):
                        nc.tensor.matmul(
                            ps[:],
                            lhsT=wq[:, 2 * k8:2 * k8 + 2, nt * P:(nt + 1) * P],
                            rhs=xq[:, 2 * k8:2 * k8 + 2, m0:m0 + 512],
                            start=(k8 == 0),
                            stop=(k8 == k_pairs - 1),
                            perf_mode=mybir.MatmulPerfMode.DoubleRow,
                        )
                    ob = ev.tile([P, 512], F16, tag="ob", name=f"ob_{mq}_{nt}")
                    nc.scalar.activation(
                        ob[:], ps[:],
                        mybir.ActivationFunctionType.Identity,
                        bias=bias32[:, nt:nt + 1],
                        scale=s2[:],
                    )
                    nc.sync.dma_start(
                        out_t.ap()[nt * P:(nt + 1) * P, m0:m0 + 512], ob[:]
                    )

    nc.compile()
    return nc


_NC_CACHE = {}


def _get_nc():
    if "nc" not in _NC_CACHE:
        _NC_CACHE["nc"] = build_nc()
    return _NC_CACHE["nc"]


def kernel(x, weight, bias):
    x = np.asarray(x, dtype=np.float16).reshape(M, K)
    weight = np.asarray(weight, dtype=np.float16)
    bias = np.asarray(bias, dtype=np.float16)

    nc = _get_nc()
    wt = np.ascontiguousarray(weight.T)  # [K, N] — static-weight layout prep
    in_maps = [
        {"x": x[c * MS:(c + 1) * MS], "wt": wt, "b": bias}
        for c in range(N_CORES)
    ]
    trace = bool(int(os.environ.get("KERNEL_TRACE", "0")))
    res = run_bass_kernel_spmd(nc, in_maps, list(range(N_CORES)), trace=trace)
    _NC_CACHE["last_result"] = res

    out = np.empty((M, N), dtype=np.float16)
    for c in range(N_CORES):
        out[c * MS:(c + 1) * MS, :] = res.results[c]["out_t"].T
    return out.reshape(B, S, N)


# revision 14
# speedup vs baseline: 1.2435x; 1.0070x over previous
"""Trainium2 Bass kernel for dynamic-scale FP8 GEMM (MixLinear):

    out = (scale_in * scale_w) * (q8(x / scale_in) @ q8(w).T) + bias
    scale_in = max|x| / 448  (global over the whole activation tensor)

Strategy (8 NeuronCores, SPMD):
  - Data-parallel over M = B*S = 16384: each core gets a 2048-row shard of x,
    full weight + bias (replicated).
  - On-device global amax: per-core abs-max reduce, then AllReduce(max).
  - TRN fp8_e4m3 saturates at +-240 (vs OCP e4m3fn's +-448), so quantize with
    a 2x scale (values land in +-224) and fold the 2x back at dequant time.
  - x and w are DMA-transposed (fp16, xbar path) into [K-partition, K/128, M|N]
    layout, quantized on-chip to fp8, and the GEMM runs in DoubleRow perf mode
    (contraction 256 per matmul).
  - PSUM is evicted with a single ScalarE activation: out = psum*2s + bias
    (output kept N-major: psum partitions = N-tile), so bias is a per-partition
    scalar.  Per-core output is [N, M_shard]; the host transposes on gather.
"""

import os
import sys

try:
    import concourse  # noqa: F401
except ImportError:  # pragma: no cover
    for _p in ("/opt/trn_rl_repo", "/root/.axon_site/_ro/trn_rl_repo"):
        if os.path.isdir(_p) and _p not in sys.path:
            sys.path.insert(0, _p)

import numpy as np

import concourse.bacc as bacc
import concourse.bass as bass  # noqa: F401
import concourse.mybir as mybir
import concourse.tile as tile
from concourse import bass_isa
from concourse.bass_utils import run_bass_kernel_spmd

# Problem shapes (hardcoded per contract).
B, S, K, N = 4, 4096, 2048, 2048
M = B * S
N_CORES = 8
MS = M // N_CORES  # 2048 rows of x per core

P = 128
F16 = mybir.dt.float16
F32 = mybir.dt.float32
FP8 = mybir.dt.float8e4


def build_nc(ms=MS, k=K, n=N, n_cores=N_CORES):
    """Build + compile the per-core Bass program (SPMD: same NEFF on all cores)."""
    ko = k // P          # k-outer planes
    assert k % 256 == 0 and ms % 1024 == 0 and n % 256 == 0
    m_chunks = ms // 512     # x transpose/quant chunk count (512 m each)
    n_chunks = n // 256      # w load/quant chunk count
    nt_tiles = n // P        # GEMM stationary n-tiles
    k_pairs = ko // 2        # DoubleRow k steps
    m_half = ms // 2
    mc512 = m_half // 512    # 512-wide m chunks per half

    nc = bacc.Bacc("TRN2", target_bir_lowering=False, debug=False, num_devices=n_cores)
    x = nc.dram_tensor("x", [ms, k], F16, kind="ExternalInput")
    wt = nc.dram_tensor("wt", [k, n], F16, kind="ExternalInput")
    b = nc.dram_tensor("b", [n], F16, kind="ExternalInput")
    out_t = nc.dram_tensor("out_t", [n, ms], F16, kind="ExternalOutput")

    with tile.TileContext(nc) as tc:
        with (
            tc.tile_pool(name="big", bufs=1) as big,
            tc.tile_pool(name="xn", bufs=3) as xnp,
            tc.tile_pool(name="small", bufs=1) as small,
            tc.tile_pool(name="ev", bufs=4) as ev,
            tc.tile_pool(name="psum", bufs=2, space="PSUM") as psum,
            tc.tile_pool(name="dram", bufs=1, space="DRAM") as dram,
        ):
            # Persistent SBUF tensors.
            xT = big.tile([P, m_chunks, ko, 512], F16)  # x^T, chunk-major
            xq = big.tile([P, ko, ms], FP8)    # quantized x (scale 2s)
            wq = big.tile([P, ko, n], FP8)     # quantized w (scale 1)

            # ---- Phase A ------------------------------------------------
            # Xbar transposes of x: one clean burst on the Sync queue.
            # (Transposes corrupt if issued from two queues concurrently,
            # and Tile serializes transpose<->copy transitions globally, so
            # every DRAM->SBUF copy and the collective are ordered after the
            # burst.)  DVE abs-max reduces trail each chunk.
            acc_cols = small.tile([P, m_chunks * 4], F32)
            tr_insts = []
            for mc in range(m_chunks):
                ti = nc.sync.dma_start(
                    out=xT[:, mc],
                    in_=x.ap()[mc * 512:(mc + 1) * 512, :],
                    transpose=True,
                )
                tr_insts.append(ti)
                nc.vector.tensor_reduce(
                    acc_cols[:, mc * 4:(mc + 1) * 4],
                    xT[:, mc].rearrange("p j f -> p (j f)").rearrange(
                        "p (a f) -> p a f", a=4
                    ),
                    axis=mybir.AxisListType.X,
                    op=mybir.AluOpType.max,
                    apply_absolute_value=True,
                )
            amax_col = small.tile([P, 1], F32)
            nc.vector.tensor_reduce(
                amax_col, acc_cols[:], axis=mybir.AxisListType.X,
                op=mybir.AluOpType.max,
            )
            amax_all = small.tile([P, 1], F32)
            nc.gpsimd.partition_all_reduce(
                amax_all, amax_col, channels=P, reduce_op=bass_isa.ReduceOp.max
            )

            # ---- AllGather amaxes across cores, reduce locally ------------
            cc_in = dram.tile([1], F32)
            cc_addr = "Shared" if n_cores > 4 else "Local"
            cc_out = dram.tile([n_cores], F32, addr_space=cc_addr)
            cci = nc.scalar.dma_start(cc_in[:], amax_all[0:1, 0])
            tile.add_dep_helper(
                cci.ins, tr_insts[-1].ins,
                reason="xbar: cc staging after transpose burst",
            )
            nc.gpsimd.collective_compute(
                "AllGather",
                mybir.AluOpType.bypass,
                replica_groups=[list(range(n_cores))],
                ins=[cc_in.opt()],
                outs=[cc_out.opt()],
            )
            # ---- Phase W: weight load+quantize via SWDGE cast-DMA ---------
            # wt DRAM is already [K, N]; SWDGE casts fp16->fp8e4 (RNE,
            # verified) during the transfer.  k = j*128 + p matches the
            # xbar-transpose k-mapping used for x.  Chunked so the GEMM can
            # start on the first n-range; ordered after the transpose burst
            # (DRAM->SBUF copies conflict with xbar-transpose mode).
            for i in range(4):
                n0 = i * (n // 4)
                wi = nc.gpsimd.dma_start(
                    out=wq[:, :, n0:n0 + n // 4],
                    in_=wt.ap()[:, n0:n0 + n // 4].rearrange(
                        "(j p) n2 -> p j n2", p=P
                    ),
                )
                tile.add_dep_helper(
                    wi.ins, tr_insts[-1].ins,
                    reason="xbar: weight copy after transpose burst",
                )

            # bias -> SBUF [128, n/128] fp32, [p, j] = bias[j*128 + p]
            bias16 = small.tile([P, nt_tiles], F16)
            bi = nc.scalar.dma_start(
                bias16[:], b.ap().rearrange("(j p) -> p j", p=P)
            )
            tile.add_dep_helper(
                bi.ins, tr_insts[-1].ins,
                reason="xbar: bias copy after transpose burst",
            )
            bias32 = small.tile([P, nt_tiles], F32)
            nc.vector.tensor_copy(bias32[:], bias16[:])

            scal0 = small.tile([P, n_cores], F32)
            nc.scalar.dma_start(scal0[0:1, :], cc_out[:])
            amax1 = small.tile([P, 1], F32)
            nc.vector.tensor_reduce(
                amax1[0:1, :], scal0[0:1, :], axis=mybir.AxisListType.X,
                op=mybir.AluOpType.max,
            )
            amax_bc = small.tile([P, 1], F32)
            nc.gpsimd.partition_broadcast(amax_bc, amax1[0:1, :], channels=P)

            # inv2s = 224/amax (quant scale), s2 = amax/224 (dequant scale)
            inv_amax = small.tile([P, 1], F32)
            nc.vector.reciprocal(inv_amax, amax_bc)
            inv2s = small.tile([P, 1], F32)
            nc.vector.tensor_scalar_mul(inv2s, inv_amax, 224.0)
            s2 = small.tile([P, 1], F32)
            nc.vector.tensor_scalar_mul(s2, amax_bc, 1.0 / 224.0)

            # ---- Phases Q+G interleaved: quantize a 512-m quarter, GEMM it.
            # Quantization alternates VectorE (tensor_scalar, ~2x mode) and
            # ScalarE (activation w/ scale) per quarter.
            for mq in range(ms // 512):
                sl = slice(mq * 512, (mq + 1) * 512)
                if mq % 2 == 0:
                    nc.vector.tensor_scalar(
                        xq[:, :, sl], xT[:, mq], inv2s[:], None,
                        mybir.AluOpType.mult,
                    )
                else:
                    nc.scalar.activation(
                        xq[:, :, sl], xT[:, mq],
                        mybir.ActivationFunctionType.Copy, scale=inv2s[:],
                    )
                m0 = mq * 512
                for nt in range(nt_tiles):
                    ps = psum.tile(
                        [P, 512], F32, tag="ps", bufs=4, name=f"ps_{mq}_{nt}"
                    )
                    for k8 in range(k_pairs):
                        nc.tensor.matmul(
                            ps[:],
                            lhsT=wq[:, 2 * k8:2 * k8 + 2, nt * P:(nt + 1) * P],
                            rhs=xq[:, 2 * k8:2 * k8 + 2, m0:m0 + 512],
                            start=(k8 == 0),
                            stop=(k8 == k_pairs - 1),
                            perf_mode=mybir.MatmulPerfMode.DoubleRow,
                        )
                    ob = ev.tile([P, 512], F16, tag="ob", name=f"ob_{mq}_{nt}")
                    nc.scalar.activation(
                        ob[:], ps[:],
                        mybir.ActivationFunctionType.Identity,
                        bias=bias32[:, nt:nt + 1],
                        scale=s2[:],
                    )
                    nc.sync.dma_start(
                        out_t.ap()[nt * P:(nt + 1) * P, m0:m0 + 512], ob[:]
                    )

    nc.compile()
    return nc


_NC_CACHE = {}


def _get_nc():
    if "nc" not in _NC_CACHE:
        _NC_CACHE["nc"] = build_nc()
    return _NC_CACHE["nc"]


def kernel(x, weight, bias):
    x = np.asarray(x, dtype=np.float16).reshape(M, K)
    weight = np.asarray(weight, dtype=np.float16)
    bias = np.asarray(bias, dtype=np.float16)

    nc = _get_nc()
    wt = np.ascontiguousarray(weight.T)  # [K, N] — static-weight layout prep
    in_maps = [
        {"x": x[c * MS:(c + 1) * MS], "wt": wt, "b": bias}
        for c in range(N_CORES)
    ]
    trace = bool(int(os.environ.get("KERNEL_TRACE", "0")))
    res = run_bass_kernel_spmd(nc, in_maps, list(range(N_CORES)), trace=trace)
    _NC_CACHE["last_result"] = res

    out = np.empty((M, N), dtype=np.float16)
    for c in range(N_CORES):
        out[c * MS:(c + 1) * MS, :] = res.results[c]["out_t"].T
    return out.reshape(B, S, N)


# revision 16
# speedup vs baseline: 1.3353x; 1.0739x over previous
"""Trainium2 Bass kernel for dynamic-scale FP8 GEMM (MixLinear):

    out = (scale_in * scale_w) * (q8(x / scale_in) @ q8(w).T) + bias
    scale_in = max|x| / 448  (global over the whole activation tensor)

Strategy (8 NeuronCores, SPMD):
  - Data-parallel over M = B*S = 16384: each core gets a 2048-row shard of x,
    full weight + bias (replicated).
  - On-device global amax: per-core abs-max reduce, then AllReduce(max).
  - TRN fp8_e4m3 saturates at +-240 (vs OCP e4m3fn's +-448), so quantize with
    a 2x scale (values land in +-224) and fold the 2x back at dequant time.
  - x and w are DMA-transposed (fp16, xbar path) into [K-partition, K/128, M|N]
    layout, quantized on-chip to fp8, and the GEMM runs in DoubleRow perf mode
    (contraction 256 per matmul).
  - PSUM is evicted with a single ScalarE activation: out = psum*2s + bias
    (output kept N-major: psum partitions = N-tile), so bias is a per-partition
    scalar.  Per-core output is [N, M_shard]; the host transposes on gather.
"""

import os
import sys

try:
    import concourse  # noqa: F401
except ImportError:  # pragma: no cover
    for _p in ("/opt/trn_rl_repo", "/root/.axon_site/_ro/trn_rl_repo"):
        if os.path.isdir(_p) and _p not in sys.path:
            sys.path.insert(0, _p)

import numpy as np

import concourse.bacc as bacc
import concourse.bass as bass  # noqa: F401
import concourse.mybir as mybir
import concourse.tile as tile
from concourse import bass_isa
from concourse.bass_utils import run_bass_kernel_spmd

# Problem shapes (hardcoded per contract).
B, S, K, N = 4, 4096, 2048, 2048
M = B * S
N_CORES = 8
MS = M // N_CORES  # 2048 rows of x per core

P = 128
F16 = mybir.dt.float16
F32 = mybir.dt.float32
FP8 = mybir.dt.float8e4


def build_nc(ms=MS, k=K, n=N, n_cores=N_CORES):
    """Build + compile the per-core Bass program (SPMD: same NEFF on all cores)."""
    ko = k // P          # k-outer planes
    assert k % 256 == 0 and ms % 1024 == 0 and n % 256 == 0
    m_chunks = ms // 512     # x transpose/quant chunk count (512 m each)
    n_chunks = n // 256      # w load/quant chunk count
    nt_tiles = n // P        # GEMM stationary n-tiles
    k_pairs = ko // 2        # DoubleRow k steps
    m_half = ms // 2
    mc512 = m_half // 512    # 512-wide m chunks per half

    nc = bacc.Bacc("TRN2", target_bir_lowering=False, debug=False, num_devices=n_cores)
    x = nc.dram_tensor("x", [ms, k], F16, kind="ExternalInput")
    wt = nc.dram_tensor("wt", [k, n], F16, kind="ExternalInput")
    b = nc.dram_tensor("b", [n], F16, kind="ExternalInput")
    out_t = nc.dram_tensor("out_t", [n, ms], F16, kind="ExternalOutput")

    with tile.TileContext(nc) as tc:
        with (
            tc.tile_pool(name="big", bufs=1) as big,
            tc.tile_pool(name="xn", bufs=3) as xnp,
            tc.tile_pool(name="small", bufs=1) as small,
            tc.tile_pool(name="ev", bufs=6) as ev,
            tc.tile_pool(name="psum", bufs=2, space="PSUM") as psum,
            tc.tile_pool(name="dram", bufs=1, space="DRAM") as dram,
        ):
            # Persistent SBUF tensors.
            xT = big.tile([P, m_chunks, ko, 512], F16)  # x^T, chunk-major
            xq = big.tile([P, ko, ms], FP8)    # quantized x (scale 2s)
            wq = big.tile([P, ko, n], FP8)     # quantized w (scale 1)

            # ---- Phase A ------------------------------------------------
            # Xbar transposes of x: one clean burst on the Sync queue.
            # (Transposes corrupt if issued from two queues concurrently,
            # and Tile serializes transpose<->copy transitions globally, so
            # every DRAM->SBUF copy and the collective are ordered after the
            # burst.)  DVE abs-max reduces trail each chunk.
            acc_cols = small.tile([P, m_chunks * 2], F32)
            tr_insts = []
            for mc in range(m_chunks):
                ti = nc.sync.dma_start(
                    out=xT[:, mc],
                    in_=x.ap()[mc * 512:(mc + 1) * 512, :],
                    transpose=True,
                )
                tr_insts.append(ti)
                for hh in range(2):
                    nc.vector.tensor_reduce(
                        acc_cols[:, mc * 2 + hh:mc * 2 + hh + 1],
                        xT[:, mc, :, hh * 256:(hh + 1) * 256],
                        axis=mybir.AxisListType.XY,
                        op=mybir.AluOpType.max,
                        apply_absolute_value=True,
                    )
            amax_col = small.tile([P, 1], F32)
            nc.vector.tensor_reduce(
                amax_col, acc_cols[:], axis=mybir.AxisListType.X,
                op=mybir.AluOpType.max,
            )
            amax_all = small.tile([P, 1], F32)
            nc.gpsimd.partition_all_reduce(
                amax_all, amax_col, channels=P, reduce_op=bass_isa.ReduceOp.max
            )

            # ---- AllGather amaxes across cores, reduce locally ------------
            cc_in = dram.tile([1], F32)
            cc_addr = "Shared" if n_cores > 4 else "Local"
            cc_out = dram.tile([n_cores], F32, addr_space=cc_addr)
            cci = nc.scalar.dma_start(cc_in[:], amax_all[0:1, 0])
            tile.add_dep_helper(
                cci.ins, tr_insts[-1].ins,
                reason="xbar: cc staging after transpose burst",
            )
            nc.gpsimd.collective_compute(
                "AllGather",
                mybir.AluOpType.bypass,
                replica_groups=[list(range(n_cores))],
                ins=[cc_in.opt()],
                outs=[cc_out.opt()],
            )
            # ---- Phase W: weight load+quantize via SWDGE cast-DMA ---------
            # wt DRAM is already [K, N]; SWDGE casts fp16->fp8e4 (RNE,
            # verified) during the transfer.  k = j*128 + p matches the
            # xbar-transpose k-mapping used for x.  Chunked so the GEMM can
            # start on the first n-range; ordered after the transpose burst
            # (DRAM->SBUF copies conflict with xbar-transpose mode).
            for i in range(4):
                n0 = i * (n // 4)
                wi = nc.gpsimd.dma_start(
                    out=wq[:, :, n0:n0 + n // 4],
                    in_=wt.ap()[:, n0:n0 + n // 4].rearrange(
                        "(j p) n2 -> p j n2", p=P
                    ),
                )
                tile.add_dep_helper(
                    wi.ins, tr_insts[-1].ins,
                    reason="xbar: weight copy after transpose burst",
                )

            # bias -> SBUF [128, n/128] fp32, [p, j] = bias[j*128 + p]
            bias16 = small.tile([P, nt_tiles], F16)
            bi = nc.scalar.dma_start(
                bias16[:], b.ap().rearrange("(j p) -> p j", p=P)
            )
            tile.add_dep_helper(
                bi.ins, tr_insts[-1].ins,
                reason="xbar: bias copy after transpose burst",
            )
            bias32 = small.tile([P, nt_tiles], F32)
            nc.vector.tensor_copy(bias32[:], bias16[:])

            scal0 = small.tile([P, n_cores], F32)
            nc.scalar.dma_start(scal0[0:1, :], cc_out[:])
            amax1 = small.tile([P, 1], F32)
            nc.vector.tensor_reduce(
                amax1[0:1, :], scal0[0:1, :], axis=mybir.AxisListType.X,
                op=mybir.AluOpType.max,
            )
            amax_bc = small.tile([P, 1], F32)
            nc.gpsimd.partition_broadcast(amax_bc, amax1[0:1, :], channels=P)

            # inv2s = 224/amax (quant scale), s2 = amax/224 (dequant scale)
            inv_amax = small.tile([P, 1], F32)
            nc.vector.reciprocal(inv_amax, amax_bc)
            inv2s = small.tile([P, 1], F32)
            nc.vector.tensor_scalar_mul(inv2s, inv_amax, 224.0)
            s2 = small.tile([P, 1], F32)
            nc.vector.tensor_scalar_mul(s2, amax_bc, 1.0 / 224.0)

            # ---- Phases Q+G interleaved: quantize a 512-m quarter, GEMM it.
            # Quantization alternates VectorE (tensor_scalar, ~2x mode) and
            # ScalarE (activation w/ scale) per quarter.
            for mq in range(ms // 512):
                sl = slice(mq * 512, (mq + 1) * 512)
                if mq % 2 == 0:
                    nc.vector.tensor_scalar(
                        xq[:, :, sl], xT[:, mq], inv2s[:], None,
                        mybir.AluOpType.mult,
                    )
                else:
                    nc.scalar.activation(
                        xq[:, :, sl], xT[:, mq],
                        mybir.ActivationFunctionType.Copy, scale=inv2s[:],
                    )
                m0 = mq * 512
                for nt in range(nt_tiles):
                    ps = psum.tile(
                        [P, 512], F32, tag="ps", bufs=6, name=f"ps_{mq}_{nt}"
                    )
                    for k8 in range(k_pairs):
                        nc.tensor.matmul(
                            ps[:],
                            lhsT=wq[:, 2 * k8:2 * k8 + 2, nt * P:(nt + 1) * P],
                            rhs=xq[:, 2 * k8:2 * k8 + 2, m0:m0 + 512],
                            start=(k8 == 0),
                            stop=(k8 == k_pairs - 1),
                            perf_mode=mybir.MatmulPerfMode.DoubleRow,
                        )
                    ob = ev.tile([P, 512], F16, tag="ob", name=f"ob_{mq}_{nt}")
                    nc.scalar.activation(
                        ob[:], ps[:],
                        mybir.ActivationFunctionType.Identity,
                        bias=bias32[:, nt:nt + 1],
                        scale=s2[:],
                    )
                    nc.sync.dma_start(
                        out_t.ap()[nt * P:(nt + 1) * P, m0:m0 + 512], ob[:]
                    )

    nc.compile()
    return nc


_NC_CACHE = {}


def _get_nc():
    if "nc" not in _NC_CACHE:
        _NC_CACHE["nc"] = build_nc()
    return _NC_CACHE["nc"]


def kernel(x, weight, bias):
    x = np.asarray(x, dtype=np.float16).reshape(M, K)
    weight = np.asarray(weight, dtype=np.float16)
    bias = np.asarray(bias, dtype=np.float16)

    nc = _get_nc()
    wt = np.ascontiguousarray(weight.T)  # [K, N] — static-weight layout prep
    in_maps = [
        {"x": x[c * MS:(c + 1) * MS], "wt": wt, "b": bias}
        for c in range(N_CORES)
    ]
    trace = bool(int(os.environ.get("KERNEL_TRACE", "0")))
    res = run_bass_kernel_spmd(nc, in_maps, list(range(N_CORES)), trace=trace)
    _NC_CACHE["last_result"] = res

    out = np.empty((M, N), dtype=np.float16)
    for c in range(N_CORES):
        out[c * MS:(c + 1) * MS, :] = res.results[c]["out_t"].T
    return out.reshape(B, S, N)
